# revision 5
# baseline (speedup 1.0000x reference)
"""Co-attention head kernel for 8 Trainium2 NeuronCores — v4.

Reference computation (H=4096, heads=4, d=1024, N=1024):
    q/k/v[h] = node1|node2 @ W{q,k,v}[h] + b        ([N, d] per head)
    r[h]     = (q[h] @ k[h]^T * 1/sqrt(d)) .* v[h]  (elementwise, N==d)
    out      = LayerNorm(concat_h r[h])             ([N, 4096])

Sharding: 8 cores = 4 heads x 2 node-halves. Core c=(h=c//2, s=c%2) owns
rows n-own = s*512:(s+1)*512 of q/scores/v for head h:
  - kT[f, m-own]  = Wk[h]^T @ n2T[:, m-own]; pair AllGather of the 1 MB
    bf16 halves gives full kT — issued right after the K projection and
    covered by the whole Q+V projections (~110 us) before scores need it
  - qT[f, n-own]  = Wq[h]^T @ n1T[:, n-own]   (scaled by 1/32 on host)
  - v[n-own, :]   = n2T[:, n-own]^T @ Wv[h]   (n2T half reused from K)
  - scores[n-own, :] = qT^T @ kT              (full contraction, no RS)
  - r = scores .* v; LayerNorm stats via AllReduce over same-parity cores
  - writes out block [512, 1024]; host assembles the [1024, 4096] output.

Collective-latency discipline (the point of v5): engine queues are strict
FIFO, so any op that waits on a collective result head-blocks its queue
and stalls the NEXT rep's work behind it (~80 us/rep measured on v2).
Two countermeasures:
  1. The kT AllGather is issued right after the K projection and first
     consumed by the scores phase, ~110 us later (Q + V projections in
     between).
  2. The AllReduce-dependent LayerNorm tail of rep i is software-pipelined:
     emitted into rep i+1's instruction stream after its Q-phase bias adds,
     ~115 us after the AllReduce was triggered. Output DMA triggers sit on
     the gpsimd queue so the sync/scalar stream queues never wait on them.
All input streams ride the sync queue; DVE does bias/stats/normalize; the
scalar engine only does the LN Sqrt.

All wire data is bf16; PSUM accumulation is fp32. NB: tensor_tensor_reduce
crashes TRN2 hw via this toolchain — use tensor_mul + tensor_reduce.
"""

from contextlib import ExitStack

import numpy as np

import concourse.bass as bass
import concourse.tile as tile
from concourse import bacc, mybir
from concourse.bass_utils import run_bass_kernel_spmd

F32 = mybir.dt.float32
BF16 = mybir.dt.bfloat16

H_DIM = 4096
N_HEADS = 4
D_HEAD = 1024
N = 1024
LN_EPS = 1e-5
N_CORES = 8
SCALE = 1.0 / 32.0  # 1/sqrt(D_HEAD)

K_TILES = H_DIM // 128  # 32
KB = 4  # k-tiles per stream DMA
NSTEP = K_TILES // KB  # 8

ALU = mybir.AluOpType
ACT_FN = mybir.ActivationFunctionType


def _bcast_ap(ap: bass.AP, parts: int = 128) -> bass.AP:
    """[n] DRAM vector viewed as [parts, n] with 0-stride partitions."""
    return bass.AP(tensor=ap.tensor, offset=ap.offset, ap=[[0, parts], *ap.ap])


def build_program(no_collectives: bool = False, reps: int = 1, wire: str = "bf16"):
    nc = bacc.Bacc("TRN2", target_bir_lowering=False, debug=False, num_devices=N_CORES)

    n1o = nc.dram_tensor("n1o", [H_DIM, 512], BF16, kind="ExternalInput").ap()
    n2o = nc.dram_tensor("n2o", [H_DIM, 512], BF16, kind="ExternalInput").ap()
    wq = nc.dram_tensor("wq", [H_DIM, D_HEAD], BF16, kind="ExternalInput").ap()
    wk = nc.dram_tensor("wk", [H_DIM, D_HEAD], BF16, kind="ExternalInput").ap()
    wv = nc.dram_tensor("wv", [H_DIM, D_HEAD], BF16, kind="ExternalInput").ap()
    bq = nc.dram_tensor("bq", [D_HEAD], F32, kind="ExternalInput").ap()
    bk = nc.dram_tensor("bk", [D_HEAD], F32, kind="ExternalInput").ap()
    bv = nc.dram_tensor("bv", [D_HEAD], F32, kind="ExternalInput").ap()
    gam = nc.dram_tensor("gam", [D_HEAD], F32, kind="ExternalInput").ap()
    bet = nc.dram_tensor("bet", [D_HEAD], F32, kind="ExternalInput").ap()
    out = nc.dram_tensor("out", [512, D_HEAD], F32, kind="ExternalOutput").ap()

    # k-tile views: [128, K_TILES, cols]; col-group a = DRAM rows a*128+p
    n1o_2 = n1o.rearrange("(a p) n -> p a n", p=128)
    n2o_2 = n2o.rearrange("(a p) n -> p a n", p=128)
    wq_2 = wq.rearrange("(a p) f -> p a f", p=128)
    wk_2 = wk.rearrange("(a p) f -> p a f", p=128)
    wv_2 = wv.rearrange("(a p) f -> p a f", p=128)

    with tile.TileContext(nc) as tc, ExitStack() as ctx:
        singles = ctx.enter_context(tc.tile_pool(name="singles", bufs=1))
        streams = ctx.enter_context(tc.tile_pool(name="streams", bufs=3))
        resident = ctx.enter_context(tc.tile_pool(name="resident", bufs=1))
        ps = ctx.enter_context(tc.tile_pool(name="ps", bufs=1, space="PSUM"))
        fin = ctx.enter_context(tc.tile_pool(name="fin", bufs=1))
        dram = ctx.enter_context(tc.tile_pool(name="dram", bufs=1, space="DRAM"))

        # ---- constants (loaded once) ----
        bq_sb = singles.tile([128, 8], F32)
        nc.sync.dma_start(out=bq_sb, in_=bq.rearrange("(b p) -> p b", p=128))
        bk_sb = singles.tile([128, 8], F32)
        nc.sync.dma_start(out=bk_sb, in_=bk.rearrange("(b p) -> p b", p=128))
        bv_b = singles.tile([128, D_HEAD], F32)
        nc.sync.dma_start(out=bv_b, in_=_bcast_ap(bv))
        gam_b = singles.tile([128, D_HEAD], F32)
        nc.sync.dma_start(out=gam_b, in_=_bcast_ap(gam))
        bet_b = singles.tile([128, D_HEAD], F32)
        nc.sync.dma_start(out=bet_b, in_=_bcast_ap(bet))
        eps_sb = singles.tile([128, 1], F32)
        nc.vector.memset(eps_sb, LN_EPS)

        def emit_rep(prev_tail):
            ag_in = dram.tile([D_HEAD, 512], BF16, name="ag_in", tag="ag_in", bufs=2)
            ag_out = dram.tile(
                [2 * D_HEAD, 512], BF16, name="ag_out", tag="ag_out", bufs=2
            )
            ar_in = dram.tile([512, 2], F32, name="ar_in", tag="ar_in", bufs=2)
            ar_out = dram.tile([512, 2], F32, name="ar_out", tag="ar_out", bufs=2)

            n2o_sb = resident.tile(
                [128, K_TILES, 512], BF16, name="n2o_sb", tag="n2o_sb", bufs=2
            )
            qT = [
                resident.tile([128, 512], BF16, name=f"qT{f}", tag=f"qT{f}")
                for f in range(8)
            ]
            kT = [
                resident.tile([128, D_HEAD], BF16, name=f"kT{f}", tag=f"kT{f}")
                for f in range(8)
            ]
            kTo = [
                resident.tile([128, 512], BF16, name=f"kTo{f}", tag=f"kTo{f}")
                for f in range(8)
            ]
            v_sb = [
                resident.tile([128, D_HEAD], F32, name=f"v{t}", tag=f"v{t}", bufs=1)
                for t in range(4)
            ]

            # n2T own half: loaded once, reused by K (moving) and V (stationary)
            for c4 in range(NSTEP):
                nc.sync.dma_start(
                    out=n2o_sb[:, KB * c4 : KB * c4 + KB, :],
                    in_=n2o_2[:, KB * c4 : KB * c4 + KB, :],
                )

            # ---- K projection: kT_own[f, m-own] = wk^T @ n2o ----
            kps = [
                ps.tile([128, 512], F32, name=f"kp{f}", tag=f"pp{f}")
                for f in range(8)
            ]
            for k4 in range(NSTEP):
                wk_t = streams.tile([128, KB, D_HEAD], BF16, name="wk_t", tag="w_t")
                nc.sync.dma_start(out=wk_t, in_=wk_2[:, KB * k4 : KB * k4 + KB, :])
                for a in range(KB):
                    first = k4 == 0 and a == 0
                    last = k4 == NSTEP - 1 and a == KB - 1
                    for f in range(8):
                        nc.tensor.matmul(
                            kps[f][:],
                            wk_t[:, a, f * 128 : (f + 1) * 128],
                            n2o_sb[:, k4 * KB + a, :],
                            start=first,
                            stop=last,
                        )
            for f in range(8):
                nc.vector.tensor_scalar(
                    out=kTo[f][:],
                    in0=kps[f][:],
                    scalar1=bk_sb[:, f : f + 1],
                    scalar2=None,
                    op0=ALU.add,
                )
                nc.gpsimd.dma_start(
                    out=ag_in[f * 128 : (f + 1) * 128, :], in_=kTo[f][:]
                )

            # pair AllGather of kT halves (even rank = m 0:512, odd = 512:1024);
            # covered by the whole Q+V projections before scores need kT
            if no_collectives:
                nc.gpsimd.dma_start(out=ag_out[0:D_HEAD, :], in_=ag_in[:])
                nc.gpsimd.dma_start(out=ag_out[D_HEAD : 2 * D_HEAD, :], in_=ag_in[:])
            else:
                nc.gpsimd.collective_compute(
                    "AllGather",
                    ALU.bypass,
                    replica_groups=[[0, 1], [2, 3], [4, 5], [6, 7]],
                    ins=[ag_in[:].opt()],
                    outs=[ag_out[:].opt()],
                )
            # assemble full kT from AllGather output (uniform layout per rank)
            for f in range(8):
                nc.gpsimd.dma_start(
                    out=kT[f][:, 0:512], in_=ag_out[f * 128 : (f + 1) * 128, :]
                )
                nc.gpsimd.dma_start(
                    out=kT[f][:, 512:1024],
                    in_=ag_out[D_HEAD + f * 128 : D_HEAD + (f + 1) * 128, :],
                )

            # ---- Q projection: qT[f, n-own] = (wq*scale)^T @ n1o ----
            qps = [
                ps.tile([128, 512], F32, name=f"qp{f}", tag=f"pp{f}")
                for f in range(8)
            ]
            for k4 in range(NSTEP):
                a_t = streams.tile([128, KB, 512], BF16, name="a_t", tag="a_t")
                nc.sync.dma_start(out=a_t, in_=n1o_2[:, KB * k4 : KB * k4 + KB, :])
                wq_t = streams.tile([128, KB, D_HEAD], BF16, name="wq_t", tag="w_t")
                nc.sync.dma_start(out=wq_t, in_=wq_2[:, KB * k4 : KB * k4 + KB, :])
                for a in range(KB):
                    first = k4 == 0 and a == 0
                    last = k4 == NSTEP - 1 and a == KB - 1
                    for f in range(8):
                        nc.tensor.matmul(
                            qps[f][:],
                            wq_t[:, a, f * 128 : (f + 1) * 128],
                            a_t[:, a, :],
                            start=first,
                            stop=last,
                        )
            for f in range(8):
                nc.vector.tensor_scalar(
                    out=qT[f][:],
                    in0=qps[f][:],
                    scalar1=bq_sb[:, f : f + 1],
                    scalar2=None,
                    op0=ALU.add,
                )

            # software-pipelined tail: emit the PREVIOUS rep's AR-dependent
            # normalize here (~115 us after its AllReduce was triggered), so
            # the DVE queue never head-blocks on collective latency
            if prev_tail is not None:
                prev_tail()

            # ---- V: v[n-own, :] = n2o^T @ wv (stationary n2o blocks) ----
            vps = [
                [
                    ps.tile([128, 512], F32, name=f"vp{t}_{j}", tag=f"pp{2 * t + j}")
                    for j in range(2)
                ]
                for t in range(4)
            ]
            for k4 in range(NSTEP):
                wv_t = streams.tile([128, KB, D_HEAD], BF16, name="wv_t", tag="w_t")
                nc.sync.dma_start(out=wv_t, in_=wv_2[:, KB * k4 : KB * k4 + KB, :])
                for a in range(KB):
                    first = k4 == 0 and a == 0
                    last = k4 == NSTEP - 1 and a == KB - 1
                    for t in range(4):
                        for j in range(2):
                            nc.tensor.matmul(
                                vps[t][j][:],
                                n2o_sb[:, k4 * KB + a, t * 128 : (t + 1) * 128],
                                wv_t[:, a, j * 512 : (j + 1) * 512],
                                start=first,
                                stop=last,
                            )
            for t in range(4):
                for j in range(2):
                    nc.vector.tensor_add(
                        out=v_sb[t][:, j * 512 : (j + 1) * 512],
                        in0=vps[t][j][:],
                        in1=bv_b[:, j * 512 : (j + 1) * 512],
                    )

            # ---- scores[n-own, :] = qT^T @ kT (full f contraction) ----
            sc_tiles = []
            for nb in range(4):
                sc_sb = fin.tile([128, N], F32, name=f"sc{nb}", tag=f"sc{nb}")
                for mh in range(2):
                    sc_ps = ps.tile(
                        [128, 512], F32, name=f"sc_ps{nb}_{mh}", tag=f"pp{2 * nb + mh}"
                    )
                    for ft in range(8):
                        nc.tensor.matmul(
                            sc_ps[:],
                            qT[ft][:, nb * 128 : (nb + 1) * 128],
                            kT[ft][:, mh * 512 : (mh + 1) * 512],
                            start=(ft == 0),
                            stop=(ft == 7),
                        )
                    nc.vector.tensor_copy(
                        out=sc_sb[:, mh * 512 : (mh + 1) * 512], in_=sc_ps[:]
                    )
                sc_tiles.append(sc_sb)

            # ---- final: r = sc .* v (in-place), LN stats on DVE ----
            st_all = fin.tile([128, 4, 2], F32, name="st_all", tag="st_all", bufs=2)
            for t in range(4):
                nc.vector.tensor_mul(
                    out=sc_tiles[t][:], in0=sc_tiles[t][:], in1=v_sb[t][:]
                )
                nc.vector.tensor_reduce(
                    out=st_all[:, t, 0:1],
                    in_=sc_tiles[t][:],
                    axis=mybir.AxisListType.X,
                    op=ALU.add,
                )
                sq_t = fin.tile([128, N], F32, name="sq_t", tag="sq_t", bufs=1)
                nc.vector.tensor_mul(out=sq_t[:], in0=sc_tiles[t][:], in1=sc_tiles[t][:])
                nc.vector.tensor_reduce(
                    out=st_all[:, t, 1:2],
                    in_=sq_t[:],
                    axis=mybir.AxisListType.X,
                    op=ALU.add,
                )

            # ---- AR-dependent tail: gpsimd queue only (plus isolated Rsqrt) ----
            # AR-side DMAs ride the scalar(ACT) queue: the gpsimd queue head-
            # blocks on the NEXT rep's AllGather (kT readbacks), which would
            # re-couple this rep's LN tail to AG latency
            ar_in_2 = ar_in[:].rearrange("(b p) c -> p b c", p=128)
            ar_out_2 = ar_out[:].rearrange("(b p) c -> p b c", p=128)
            nc.scalar.dma_start(out=ar_in_2, in_=st_all)
            if no_collectives:
                nc.gpsimd.dma_start(out=ar_out[:], in_=ar_in[:])
            else:
                nc.gpsimd.collective_compute(
                    "AllReduce",
                    ALU.add,
                    replica_groups=[[0, 2, 4, 6], [1, 3, 5, 7]],
                    ins=[ar_in[:].opt()],
                    outs=[ar_out[:].opt()],
                )
            def tail():
                tot_all = fin.tile(
                    [128, 4, 2], F32, name="tot_all", tag="tot_all", bufs=2
                )
                nc.scalar.dma_start(out=tot_all, in_=ar_out_2)
                inv_h = 1.0 / float(H_DIM)
                for t in range(4):
                    mu = fin.tile([128, 1], F32, name=f"mu{t}", tag=f"mu{t}", bufs=2)
                    nc.vector.tensor_scalar_mul(
                        out=mu, in0=tot_all[:, t, 0:1], scalar1=inv_h
                    )
                    msq = fin.tile([128, 1], F32, name=f"msq{t}", tag=f"msq{t}", bufs=2)
                    nc.vector.tensor_mul(out=msq, in0=mu, in1=mu)
                    var = fin.tile([128, 1], F32, name=f"var{t}", tag=f"var{t}", bufs=2)
                    nc.vector.tensor_scalar(
                        out=var,
                        in0=tot_all[:, t, 1:2],
                        scalar1=inv_h,
                        scalar2=msq[:, 0:1],
                        op0=ALU.mult,
                        op1=ALU.subtract,
                    )
                    sd = fin.tile([128, 1], F32, name=f"sd{t}", tag=f"sd{t}", bufs=2)
                    nc.scalar.activation(
                        out=sd, in_=var, func=ACT_FN.Sqrt, bias=eps_sb[:], scale=1.0
                    )
                    inv = fin.tile([128, 1], F32, name=f"inv{t}", tag=f"inv{t}", bufs=2)
                    nc.vector.reciprocal(out=inv, in_=sd)
                    o_t = fin.tile([128, N], F32, name="o_t", tag="o_t", bufs=2)[:]
                    nc.vector.tensor_scalar(
                        out=o_t,
                        in0=sc_tiles[t][:],
                        scalar1=mu[:, 0:1],
                        scalar2=inv[:, 0:1],
                        op0=ALU.subtract,
                        op1=ALU.mult,
                    )
                    nc.vector.tensor_mul(out=o_t, in0=o_t, in1=gam_b[:])
                    nc.vector.tensor_add(out=o_t, in0=o_t, in1=bet_b[:])
                    nc.scalar.dma_start(out=out[t * 128 : (t + 1) * 128, :], in_=o_t)

            return tail

        tail = None
        for _ in range(reps):
            tail = emit_rep(tail)
        tail()

    nc.compile()
    return nc


_NC = None


def _get_program():
    global _NC
    if _NC is None:
        _NC = build_program()
    return _NC


def make_in_maps(node1, node2, Wq, bq, Wk, bk, Wv, bv, gamma, beta, wire: str = "bf16"):
    import ml_dtypes

    f32 = np.float32
    wd = ml_dtypes.bfloat16
    n1t = np.ascontiguousarray(np.asarray(node1).T).astype(wd)
    n2t = np.ascontiguousarray(np.asarray(node2).T).astype(wd)
    in_maps = []
    for c in range(N_CORES):
        h, s = c // 2, c % 2
        nsl = slice(s * 512, (s + 1) * 512)
        in_maps.append(
            {
                "n1o": np.ascontiguousarray(n1t[:, nsl]),
                "n2o": np.ascontiguousarray(n2t[:, nsl]),
                "wq": np.ascontiguousarray(Wq[h] * SCALE).astype(wd),
                "wk": np.ascontiguousarray(Wk[h]).astype(wd),
                "wv": np.ascontiguousarray(Wv[h]).astype(wd),
                "bq": np.ascontiguousarray(bq[h] * SCALE, dtype=f32),
                "bk": np.ascontiguousarray(bk[h], dtype=f32),
                "bv": np.ascontiguousarray(bv[h], dtype=f32),
                "gam": np.ascontiguousarray(gamma[h * 1024 : (h + 1) * 1024], dtype=f32),
                "bet": np.ascontiguousarray(beta[h * 1024 : (h + 1) * 1024], dtype=f32),
            }
        )
    return in_maps


def assemble(results):
    out = np.empty((N, H_DIM), np.float32)
    for c in range(N_CORES):
        h, s = c // 2, c % 2
        out[s * 512 : (s + 1) * 512, h * 1024 : (h + 1) * 1024] = results[c]["out"]
    return out


def kernel(node1, node2, Wq, bq, Wk, bk, Wv, bv, gamma, beta):
    nc = _get_program()
    in_maps = make_in_maps(node1, node2, Wq, bq, Wk, bk, Wv, bv, gamma, beta)
    res = run_bass_kernel_spmd(nc, in_maps, list(range(N_CORES)))
    return assemble(res.results)


# revision 6
# speedup vs baseline: 1.6280x; 1.6280x over previous
"""Co-attention head kernel for 8 Trainium2 NeuronCores — v4.

Reference computation (H=4096, heads=4, d=1024, N=1024):
    q/k/v[h] = node1|node2 @ W{q,k,v}[h] + b        ([N, d] per head)
    r[h]     = (q[h] @ k[h]^T * 1/sqrt(d)) .* v[h]  (elementwise, N==d)
    out      = LayerNorm(concat_h r[h])             ([N, 4096])

Sharding: 8 cores = 4 heads x 2 node-halves. Core c=(h=c//2, s=c%2) owns
rows n-own = s*512:(s+1)*512 of q/scores/v for head h:
  - kT[f, m-own]  = Wk[h]^T @ n2T[:, m-own]; pair AllGather of the 1 MB
    bf16 halves gives full kT — issued right after the K projection and
    covered by the whole Q+V projections (~110 us) before scores need it
  - qT[f, n-own]  = Wq[h]^T @ n1T[:, n-own]   (scaled by 1/32 on host)
  - v[n-own, :]   = n2T[:, n-own]^T @ Wv[h]   (n2T half reused from K)
  - scores[n-own, :] = qT^T @ kT              (full contraction, no RS)
  - r = scores .* v; LayerNorm stats via AllReduce over same-parity cores
  - writes out block [512, 1024]; host assembles the [1024, 4096] output.

Collective-latency discipline (the point of v5): engine queues are strict
FIFO, so any op that waits on a collective result head-blocks its queue
and stalls the NEXT rep's work behind it (~80 us/rep measured on v2).
Two countermeasures:
  1. The kT AllGather is issued right after the K projection and first
     consumed by the scores phase, ~110 us later (Q + V projections in
     between).
  2. The AllReduce-dependent LayerNorm tail of rep i is software-pipelined:
     emitted into rep i+1's instruction stream after its Q-phase bias adds,
     ~115 us after the AllReduce was triggered. Output DMA triggers sit on
     the gpsimd queue so the sync/scalar stream queues never wait on them.
All input streams ride the sync queue; DVE does bias/stats/normalize; the
scalar engine only does the LN Sqrt.

All wire data is bf16; PSUM accumulation is fp32. NB: tensor_tensor_reduce
crashes TRN2 hw via this toolchain — use tensor_mul + tensor_reduce.
"""

from contextlib import ExitStack

import numpy as np

import concourse.bass as bass
import concourse.tile as tile
from concourse import bacc, mybir
from concourse.bass_utils import run_bass_kernel_spmd

F32 = mybir.dt.float32
BF16 = mybir.dt.bfloat16

H_DIM = 4096
N_HEADS = 4
D_HEAD = 1024
N = 1024
LN_EPS = 1e-5
N_CORES = 8
SCALE = 1.0 / 32.0  # 1/sqrt(D_HEAD)

K_TILES = H_DIM // 128  # 32
KB = 4  # k-tiles per stream DMA
NSTEP = K_TILES // KB  # 8

ALU = mybir.AluOpType
ACT_FN = mybir.ActivationFunctionType


def _bcast_ap(ap: bass.AP, parts: int = 128) -> bass.AP:
    """[n] DRAM vector viewed as [parts, n] with 0-stride partitions."""
    return bass.AP(tensor=ap.tensor, offset=ap.offset, ap=[[0, parts], *ap.ap])


def build_program(no_collectives: bool = False, reps: int = 1, wire: str = "bf16"):
    nc = bacc.Bacc("TRN2", target_bir_lowering=False, debug=False, num_devices=N_CORES)

    n1o = nc.dram_tensor("n1o", [H_DIM, 512], BF16, kind="ExternalInput").ap()
    n2o = nc.dram_tensor("n2o", [H_DIM, 512], BF16, kind="ExternalInput").ap()
    wq = nc.dram_tensor("wq", [H_DIM, D_HEAD], BF16, kind="ExternalInput").ap()
    wk = nc.dram_tensor("wk", [H_DIM, D_HEAD], BF16, kind="ExternalInput").ap()
    wv = nc.dram_tensor("wv", [H_DIM, D_HEAD], BF16, kind="ExternalInput").ap()
    bq = nc.dram_tensor("bq", [D_HEAD], F32, kind="ExternalInput").ap()
    bk = nc.dram_tensor("bk", [D_HEAD], F32, kind="ExternalInput").ap()
    bv = nc.dram_tensor("bv", [D_HEAD], F32, kind="ExternalInput").ap()
    gam = nc.dram_tensor("gam", [D_HEAD], F32, kind="ExternalInput").ap()
    bet = nc.dram_tensor("bet", [D_HEAD], F32, kind="ExternalInput").ap()
    out = nc.dram_tensor("out", [512, D_HEAD], F32, kind="ExternalOutput").ap()

    # k-tile views: [128, K_TILES, cols]; col-group a = DRAM rows a*128+p
    n1o_2 = n1o.rearrange("(a p) n -> p a n", p=128)
    n2o_2 = n2o.rearrange("(a p) n -> p a n", p=128)
    wq_2 = wq.rearrange("(a p) f -> p a f", p=128)
    wk_2 = wk.rearrange("(a p) f -> p a f", p=128)
    wv_2 = wv.rearrange("(a p) f -> p a f", p=128)

    with tile.TileContext(nc) as tc, ExitStack() as ctx:
        singles = ctx.enter_context(tc.tile_pool(name="singles", bufs=1))
        streams = ctx.enter_context(tc.tile_pool(name="streams", bufs=3))
        resident = ctx.enter_context(tc.tile_pool(name="resident", bufs=1))
        ps = ctx.enter_context(tc.tile_pool(name="ps", bufs=1, space="PSUM"))
        fin = ctx.enter_context(tc.tile_pool(name="fin", bufs=1))
        dram = ctx.enter_context(tc.tile_pool(name="dram", bufs=1, space="DRAM"))

        # ---- constants (loaded once) ----
        bq_sb = singles.tile([128, 8], F32)
        nc.sync.dma_start(out=bq_sb, in_=bq.rearrange("(b p) -> p b", p=128))
        bk_sb = singles.tile([128, 8], F32)
        nc.sync.dma_start(out=bk_sb, in_=bk.rearrange("(b p) -> p b", p=128))
        bv_b = singles.tile([128, D_HEAD], F32)
        nc.sync.dma_start(out=bv_b, in_=_bcast_ap(bv))
        gam_b = singles.tile([128, D_HEAD], F32)
        nc.sync.dma_start(out=gam_b, in_=_bcast_ap(gam))
        bet_b = singles.tile([128, D_HEAD], F32)
        nc.sync.dma_start(out=bet_b, in_=_bcast_ap(bet))
        eps_sb = singles.tile([128, 1], F32)
        nc.vector.memset(eps_sb, LN_EPS)

        def emit_rep(prev_tail):
            ag_in = dram.tile([D_HEAD, 512], BF16, name="ag_in", tag="ag_in", bufs=2)
            ag_out = dram.tile(
                [2 * D_HEAD, 512], BF16, name="ag_out", tag="ag_out", bufs=2
            )
            ar_in = dram.tile([512, 2], F32, name="ar_in", tag="ar_in", bufs=2)
            ar_out = dram.tile([512, 2], F32, name="ar_out", tag="ar_out", bufs=2)

            n2o_sb = resident.tile(
                [128, K_TILES, 512], BF16, name="n2o_sb", tag="n2o_sb", bufs=2
            )
            qT = [
                resident.tile([128, 512], BF16, name=f"qT{f}", tag=f"qT{f}")
                for f in range(8)
            ]
            kT = [
                resident.tile([128, D_HEAD], BF16, name=f"kT{f}", tag=f"kT{f}")
                for f in range(8)
            ]
            kTo = [
                resident.tile([128, 512], BF16, name=f"kTo{f}", tag=f"kTo{f}")
                for f in range(8)
            ]
            v_sb = [
                resident.tile([128, D_HEAD], F32, name=f"v{t}", tag=f"v{t}", bufs=1)
                for t in range(4)
            ]

            # n2T own half: loaded once, reused by K (moving) and V (stationary)
            for c4 in range(NSTEP):
                nc.sync.dma_start(
                    out=n2o_sb[:, KB * c4 : KB * c4 + KB, :],
                    in_=n2o_2[:, KB * c4 : KB * c4 + KB, :],
                )

            # ---- K projection: kT_own[f, m-own] = wk^T @ n2o ----
            kps = [
                ps.tile([128, 512], F32, name=f"kp{f}", tag=f"pp{f}")
                for f in range(8)
            ]
            for k4 in range(NSTEP):
                wk_t = streams.tile([128, KB, D_HEAD], BF16, name="wk_t", tag="w_t")
                nc.sync.dma_start(out=wk_t, in_=wk_2[:, KB * k4 : KB * k4 + KB, :])
                for a in range(KB):
                    first = k4 == 0 and a == 0
                    last = k4 == NSTEP - 1 and a == KB - 1
                    for f in range(8):
                        nc.tensor.matmul(
                            kps[f][:],
                            wk_t[:, a, f * 128 : (f + 1) * 128],
                            n2o_sb[:, k4 * KB + a, :],
                            start=first,
                            stop=last,
                        )
            for f in range(8):
                nc.vector.tensor_scalar(
                    out=kTo[f][:],
                    in0=kps[f][:],
                    scalar1=bk_sb[:, f : f + 1],
                    scalar2=None,
                    op0=ALU.add,
                )
                nc.gpsimd.dma_start(
                    out=ag_in[f * 128 : (f + 1) * 128, :], in_=kTo[f][:]
                )

            # pair AllGather of kT halves (even rank = m 0:512, odd = 512:1024);
            # covered by the whole Q+V projections before scores need kT
            if no_collectives:
                nc.gpsimd.dma_start(out=ag_out[0:D_HEAD, :], in_=ag_in[:])
                nc.gpsimd.dma_start(out=ag_out[D_HEAD : 2 * D_HEAD, :], in_=ag_in[:])
            else:
                nc.gpsimd.collective_compute(
                    "AllGather",
                    ALU.bypass,
                    replica_groups=[[0, 1], [2, 3], [4, 5], [6, 7]],
                    ins=[ag_in[:].opt()],
                    outs=[ag_out[:].opt()],
                )
            # assemble full kT from AllGather output (uniform layout per rank)
            for f in range(8):
                nc.gpsimd.dma_start(
                    out=kT[f][:, 0:512], in_=ag_out[f * 128 : (f + 1) * 128, :]
                )
                nc.gpsimd.dma_start(
                    out=kT[f][:, 512:1024],
                    in_=ag_out[D_HEAD + f * 128 : D_HEAD + (f + 1) * 128, :],
                )

            # ---- Q projection: qT[f, n-own] = (wq*scale)^T @ n1o ----
            qps = [
                ps.tile([128, 512], F32, name=f"qp{f}", tag=f"pp{f}")
                for f in range(8)
            ]
            for k4 in range(NSTEP):
                a_t = streams.tile([128, KB, 512], BF16, name="a_t", tag="a_t")
                nc.sync.dma_start(out=a_t, in_=n1o_2[:, KB * k4 : KB * k4 + KB, :])
                wq_t = streams.tile([128, KB, D_HEAD], BF16, name="wq_t", tag="w_t")
                nc.sync.dma_start(out=wq_t, in_=wq_2[:, KB * k4 : KB * k4 + KB, :])
                for a in range(KB):
                    first = k4 == 0 and a == 0
                    last = k4 == NSTEP - 1 and a == KB - 1
                    for f in range(8):
                        nc.tensor.matmul(
                            qps[f][:],
                            wq_t[:, a, f * 128 : (f + 1) * 128],
                            a_t[:, a, :],
                            start=first,
                            stop=last,
                        )
            for f in range(8):
                nc.vector.tensor_scalar(
                    out=qT[f][:],
                    in0=qps[f][:],
                    scalar1=bq_sb[:, f : f + 1],
                    scalar2=None,
                    op0=ALU.add,
                )

            # software-pipelined tail: emit the PREVIOUS rep's AR-dependent
            # normalize here (~115 us after its AllReduce was triggered), so
            # the DVE queue never head-blocks on collective latency
            if prev_tail is not None:
                prev_tail()

            # ---- V: v[n-own, :] = n2o^T @ wv (stationary n2o blocks) ----
            vps = [
                [
                    ps.tile([128, 512], F32, name=f"vp{t}_{j}", tag=f"pp{2 * t + j}")
                    for j in range(2)
                ]
                for t in range(4)
            ]
            for k4 in range(NSTEP):
                wv_t = streams.tile([128, KB, D_HEAD], BF16, name="wv_t", tag="w_t")
                nc.sync.dma_start(out=wv_t, in_=wv_2[:, KB * k4 : KB * k4 + KB, :])
                for a in range(KB):
                    first = k4 == 0 and a == 0
                    last = k4 == NSTEP - 1 and a == KB - 1
                    for t in range(4):
                        for j in range(2):
                            nc.tensor.matmul(
                                vps[t][j][:],
                                n2o_sb[:, k4 * KB + a, t * 128 : (t + 1) * 128],
                                wv_t[:, a, j * 512 : (j + 1) * 512],
                                start=first,
                                stop=last,
                            )
            for t in range(4):
                for j in range(2):
                    nc.vector.tensor_add(
                        out=v_sb[t][:, j * 512 : (j + 1) * 512],
                        in0=vps[t][j][:],
                        in1=bv_b[:, j * 512 : (j + 1) * 512],
                    )

            # ---- scores[n-own, :] = qT^T @ kT (full f contraction) ----
            sc_tiles = []
            for nb in range(4):
                sc_sb = fin.tile([128, N], F32, name=f"sc{nb}", tag=f"sc{nb}")
                for mh in range(2):
                    sc_ps = ps.tile(
                        [128, 512], F32, name=f"sc_ps{nb}_{mh}", tag=f"pp{2 * nb + mh}"
                    )
                    for ft in range(8):
                        nc.tensor.matmul(
                            sc_ps[:],
                            qT[ft][:, nb * 128 : (nb + 1) * 128],
                            kT[ft][:, mh * 512 : (mh + 1) * 512],
                            start=(ft == 0),
                            stop=(ft == 7),
                        )
                    nc.vector.tensor_copy(
                        out=sc_sb[:, mh * 512 : (mh + 1) * 512], in_=sc_ps[:]
                    )
                sc_tiles.append(sc_sb)

            # ---- final: r = sc .* v (in-place), LN stats on DVE ----
            st_all = fin.tile([128, 4, 2], F32, name="st_all", tag="st_all", bufs=2)
            for t in range(4):
                nc.vector.tensor_mul(
                    out=sc_tiles[t][:], in0=sc_tiles[t][:], in1=v_sb[t][:]
                )
                nc.vector.tensor_reduce(
                    out=st_all[:, t, 0:1],
                    in_=sc_tiles[t][:],
                    axis=mybir.AxisListType.X,
                    op=ALU.add,
                )
                sq_t = fin.tile([128, N], F32, name="sq_t", tag="sq_t", bufs=1)
                nc.vector.tensor_mul(out=sq_t[:], in0=sc_tiles[t][:], in1=sc_tiles[t][:])
                nc.vector.tensor_reduce(
                    out=st_all[:, t, 1:2],
                    in_=sq_t[:],
                    axis=mybir.AxisListType.X,
                    op=ALU.add,
                )

            # ---- AR-dependent tail: gpsimd queue only (plus isolated Rsqrt) ----
            ar_in_2 = ar_in[:].rearrange("(b p) c -> p b c", p=128)
            ar_out_2 = ar_out[:].rearrange("(b p) c -> p b c", p=128)
            nc.gpsimd.dma_start(out=ar_in_2, in_=st_all)
            if no_collectives:
                nc.gpsimd.dma_start(out=ar_out[:], in_=ar_in[:])
            else:
                nc.gpsimd.collective_compute(
                    "AllReduce",
                    ALU.add,
                    replica_groups=[[0, 2, 4, 6], [1, 3, 5, 7]],
                    ins=[ar_in[:].opt()],
                    outs=[ar_out[:].opt()],
                )
            def tail():
                tot_all = fin.tile(
                    [128, 4, 2], F32, name="tot_all", tag="tot_all", bufs=2
                )
                nc.gpsimd.dma_start(out=tot_all, in_=ar_out_2)
                inv_h = 1.0 / float(H_DIM)
                for t in range(4):
                    mu = fin.tile([128, 1], F32, name=f"mu{t}", tag=f"mu{t}", bufs=2)
                    nc.vector.tensor_scalar_mul(
                        out=mu, in0=tot_all[:, t, 0:1], scalar1=inv_h
                    )
                    msq = fin.tile([128, 1], F32, name=f"msq{t}", tag=f"msq{t}", bufs=2)
                    nc.vector.tensor_mul(out=msq, in0=mu, in1=mu)
                    var = fin.tile([128, 1], F32, name=f"var{t}", tag=f"var{t}", bufs=2)
                    nc.vector.tensor_scalar(
                        out=var,
                        in0=tot_all[:, t, 1:2],
                        scalar1=inv_h,
                        scalar2=msq[:, 0:1],
                        op0=ALU.mult,
                        op1=ALU.subtract,
                    )
                    sd = fin.tile([128, 1], F32, name=f"sd{t}", tag=f"sd{t}", bufs=2)
                    nc.scalar.activation(
                        out=sd, in_=var, func=ACT_FN.Sqrt, bias=eps_sb[:], scale=1.0
                    )
                    inv = fin.tile([128, 1], F32, name=f"inv{t}", tag=f"inv{t}", bufs=2)
                    nc.vector.reciprocal(out=inv, in_=sd)
                    o_t = fin.tile([128, N], F32, name="o_t", tag="o_t", bufs=2)[:]
                    nc.vector.tensor_scalar(
                        out=o_t,
                        in0=sc_tiles[t][:],
                        scalar1=mu[:, 0:1],
                        scalar2=inv[:, 0:1],
                        op0=ALU.subtract,
                        op1=ALU.mult,
                    )
                    nc.vector.tensor_mul(out=o_t, in0=o_t, in1=gam_b[:])
                    nc.vector.tensor_add(out=o_t, in0=o_t, in1=bet_b[:])
                    nc.gpsimd.dma_start(out=out[t * 128 : (t + 1) * 128, :], in_=o_t)

            return tail

        tail = None
        for _ in range(reps):
            tail = emit_rep(tail)
        tail()

    nc.compile()
    return nc


_NC = None


def _get_program():
    global _NC
    if _NC is None:
        _NC = build_program()
    return _NC


def make_in_maps(node1, node2, Wq, bq, Wk, bk, Wv, bv, gamma, beta, wire: str = "bf16"):
    import ml_dtypes

    f32 = np.float32
    wd = ml_dtypes.bfloat16
    n1t = np.ascontiguousarray(np.asarray(node1).T).astype(wd)
    n2t = np.ascontiguousarray(np.asarray(node2).T).astype(wd)
    in_maps = []
    for c in range(N_CORES):
        h, s = c // 2, c % 2
        nsl = slice(s * 512, (s + 1) * 512)
        in_maps.append(
            {
                "n1o": np.ascontiguousarray(n1t[:, nsl]),
                "n2o": np.ascontiguousarray(n2t[:, nsl]),
                "wq": np.ascontiguousarray(Wq[h] * SCALE).astype(wd),
                "wk": np.ascontiguousarray(Wk[h]).astype(wd),
                "wv": np.ascontiguousarray(Wv[h]).astype(wd),
                "bq": np.ascontiguousarray(bq[h] * SCALE, dtype=f32),
                "bk": np.ascontiguousarray(bk[h], dtype=f32),
                "bv": np.ascontiguousarray(bv[h], dtype=f32),
                "gam": np.ascontiguousarray(gamma[h * 1024 : (h + 1) * 1024], dtype=f32),
                "bet": np.ascontiguousarray(beta[h * 1024 : (h + 1) * 1024], dtype=f32),
            }
        )
    return in_maps


def assemble(results):
    out = np.empty((N, H_DIM), np.float32)
    for c in range(N_CORES):
        h, s = c // 2, c % 2
        out[s * 512 : (s + 1) * 512, h * 1024 : (h + 1) * 1024] = results[c]["out"]
    return out


def kernel(node1, node2, Wq, bq, Wk, bk, Wv, bv, gamma, beta):
    nc = _get_program()
    in_maps = make_in_maps(node1, node2, Wq, bq, Wk, bk, Wv, bv, gamma, beta)
    res = run_bass_kernel_spmd(nc, in_maps, list(range(N_CORES)))
    return assemble(res.results)
